# revision 12
# baseline (speedup 1.0000x reference)
"""AttentionBlock (B=4, C=256, H=W=64) on 8 Trainium2 NeuronCores.

Sharding: data-parallel over (batch, query-half): core i handles batch i//2,
query pixels [half*2048, (half+1)*2048), half = i%2. GroupNorm stats + k/vT
are computed per batch element (duplicated across the pair, cheap); the
O(N^2) attention work is fully sharded 8 ways. No collectives.

v2: all heavy matmuls run in fp8e4 (e4m3) with DoubleRow perf mode: one
matmul contracts K=256 (two 128-partition k-tiles interleaved along a
size-2 free dim) at 0.5 PE cycles per output row -- ~4x the f32r rate.
  1. GroupNorm stats via bn_stats/bn_aggr on the fp8 x copy + tiny f32r
     matmuls with 0/1 group matrices; rstd via ACT Sqrt + DVE reciprocal.
  2. GN fold: qkv conv weights are scaled on device (W' = W .* scale_c,
     cast to fp8); effective channel biases from tiny f32r matmuls on the
     unscaled f32r weights; v-bias pre-broadcast along free dim.
  3. q/k in [c_lo, c_hi, n] fp8 layout (DR-ready); vT[m, c] fp8.
  4. Attention per 512-query chunk, 16 key-block PAIRS (2x128 keys):
     S^T[mb, n] one DR matmul each into a 2-bank PSUM pair tile;
     E = exp(S/16 - 1) as ONE ACT instr per pair ([128,2,512], fp8 out;
     the -1 shift cancels in softmax and keeps E < 100 << fp8e4 max 240);
     O[c, n] += vT-pair^T E (DR, PSUM accum); R[n] += ones^T E (DR M=1).
     proj (fp8 DR) runs on O directly (linear, /R commutes); Rinv = DVE
     reciprocal, partition-broadcast by a K=1 f32r matmul;
     out = proj(O)*Rinv + xh' (xh' = x_half + folded biases, fp32).
All PSUM epilogues run on DVE to keep ACT free for the exp stream.
"""

import numpy as np

B, C, HW = 4, 256, 4096
NH = 2048            # query pixels per core
G, CPG = 32, 8       # groups, channels per group
EPS = 1e-5
MB = HW // 128       # 32 key blocks
NP = MB // 2         # 16 key-block pairs

_cache = {}


def build_nc():
    """Build (and cache) the Bass module."""
    if "nc" in _cache:
        return _cache["nc"]
    import concourse.tile as tile
    from concourse import bacc, mybir

    f32 = mybir.dt.float32
    f32r = mybir.dt.float32r
    fp8 = mybir.dt.float8e4
    AF = mybir.ActivationFunctionType
    OP = mybir.AluOpType
    DR = mybir.MatmulPerfMode.DoubleRow

    nc = bacc.Bacc("TRN2", target_bir_lowering=False, debug=False,
                   enable_asserts=False, num_devices=8)

    # ---- DRAM I/O (host preps everything into device layout) ----
    d_x8 = nc.dram_tensor("x8", [128, 2, HW], fp8, kind="ExternalInput")
    d_x8h = nc.dram_tensor("x8h", [128, 2, NH], fp8, kind="ExternalInput")
    d_xh = nc.dram_tensor("xh", [128, 2, NH], f32, kind="ExternalInput")
    d_wq = nc.dram_tensor("wq", [128, 2, C], f32r, kind="ExternalInput")
    d_wk = nc.dram_tensor("wk", [128, 2, C], f32r, kind="ExternalInput")
    d_wv = nc.dram_tensor("wv", [128, 2, C], f32r, kind="ExternalInput")
    d_wp8 = nc.dram_tensor("wp8", [128, 2, C], fp8, kind="ExternalInput")
    d_sb = nc.dram_tensor("sb", [128, 2, 5], f32, kind="ExternalInput")
    d_ag = nc.dram_tensor("ag", [128, 2, G], f32, kind="ExternalInput")
    d_bg = nc.dram_tensor("bg", [G, 2, 128], f32, kind="ExternalInput")
    d_out = nc.dram_tensor("out", [128, 2, NH], f32, kind="ExternalOutput")

    with tile.TileContext(nc) as tc:
        with (
            tc.tile_pool(name="big", bufs=1) as big,
            tc.tile_pool(name="cst", bufs=1) as cst,
            tc.tile_pool(name="wrk", bufs=2) as wrk,
            tc.tile_pool(name="epool", bufs=4) as epool,
            tc.tile_pool(name="gnp", bufs=1) as gnp,
            tc.tile_pool(name="ps_s", bufs=1, space="PSUM") as ps_s,
            tc.tile_pool(name="ps_o", bufs=1, space="PSUM") as ps_o,
            tc.tile_pool(name="ps_t", bufs=1, space="PSUM") as ps_t,
        ):
            # ---- ACT table warm: sqrt first (GN needs it soon); exp is
            # re-warmed after the GN chain, hidden behind the convs.
            warm = cst.tile([1, 2], f32, tag="warm")
            nc.vector.memset(warm, 1.0)
            nc.scalar.activation(out=warm[:, 1:2], in_=warm[:, 1:2],
                                 func=AF.Sqrt)
            smalls = cst.tile([128, 2, 5], f32, tag="smalls")
            nc.scalar.dma_start(out=smalls, in_=d_sb.ap())
            qb = smalls[:, :, 0:1]
            kb = smalls[:, :, 1:2]
            gb = smalls[:, :, 3:4]
            rbias = smalls[:, :, 4:5]
            ag = cst.tile([128, 2, G], f32, tag="ag")
            nc.scalar.dma_start(out=ag, in_=d_ag.ap())
            bg = cst.tile([G, 2, 128], f32, tag="bg")
            nc.scalar.dma_start(out=bg, in_=d_bg.ap())

            # ---- input loads ----
            x8 = big.tile([128, 2, HW], fp8, tag="x8")
            for ci in range(2):
                for j in range(8):
                    sl = slice(j * 512, (j + 1) * 512)
                    eng = nc.sync if (j % 2 == 0) else nc.scalar
                    eng.dma_start(out=x8[:, ci, sl], in_=d_x8.ap()[:, ci, sl])
            x8h = big.tile([128, 2, NH], fp8, tag="x8h")
            for ci in range(2):
                nc.sync.dma_start(out=x8h[:, ci, :], in_=d_x8h.ap()[:, ci, :])
            wall = cst.tile([128, 2, 3 * C], f32r, tag="wall")
            for i, d in enumerate((d_wq, d_wk, d_wv)):
                nc.scalar.dma_start(out=wall[:, :, i * C:(i + 1) * C], in_=d.ap())
            wp8 = cst.tile([128, 2, C], fp8, tag="wp8")
            nc.scalar.dma_start(out=wp8, in_=d_wp8.ap())
            xh = big.tile([128, 2, NH], f32, tag="xh")
            for ci in range(2):
                for j in range(2):
                    sl = slice(j * 1024, (j + 1) * 1024)
                    nc.sync.dma_start(out=xh[:, ci, sl], in_=d_xh.ap()[:, ci, sl])

            onesc = cst.tile([128, 2], f32, tag="onesc")
            nc.vector.memset(onesc, 1.0)
            epst = cst.tile([G, 1], f32, tag="epst")
            nc.vector.memset(epst, EPS)
            # R lhsT (DR): pair-dim stride must be 16-element aligned
            ones21t = cst.tile([128, 2, 16], fp8, tag="ones21")
            nc.vector.memset(ones21t, 1.0)
            ones21 = ones21t[:, :, 0:1]
            negone = cst.tile([128, 1], f32, tag="negone")  # exp shift
            nc.vector.memset(negone, -2.5)
            onesr = cst.tile([1, 128], f32, tag="onesr")
            nc.vector.memset(onesr, 1.0)
            ones_row = cst.tile([1, 128], f32r, tag="ones_row")  # bcast lhsT
            nc.vector.tensor_copy(out=ones_row, in_=onesr)
            # O is cast to fp8 scaled by 1/64 (O can exceed fp8e4 max 240);
            # the x64 is folded back via the R-broadcast (row of 64s).
            r64s = cst.tile([1, 128], f32, tag="r64s")
            nc.vector.memset(r64s, 64.0)
            row64 = cst.tile([1, 128], f32r, tag="row64")
            nc.vector.tensor_copy(out=row64, in_=r64s)
            inv64 = cst.tile([128, 1], f32, tag="inv64")
            nc.vector.memset(inv64, 1.0 / 64.0)

            # ---- GroupNorm stats (on the fp8 x copy; quantization noise in
            # the stats is ~1e-3 relative, far below the error budget) ----
            bstat = gnp.tile([128, 2, 8, 6], f32, tag="bstat")
            for ci in range(2):
                for j in range(8):
                    nc.vector.bn_stats(
                        out=bstat[:, ci, j, :],
                        in_=x8[:, ci, j * 512:(j + 1) * 512])
            stats2 = gnp.tile([128, 2, 2], f32, tag="stats2")  # (mean, E[x^2])
            tmp1 = gnp.tile([128, 1], f32, tag="tmp1")
            for ci in range(2):
                nc.vector.bn_aggr(out=stats2[:, ci, :], in_=bstat[:, ci, :, :])
                nc.vector.tensor_tensor(
                    out=tmp1, in0=stats2[:, ci, 0:1], in1=stats2[:, ci, 0:1],
                    op=OP.mult)
                nc.vector.tensor_tensor(
                    out=stats2[:, ci, 1:2], in0=stats2[:, ci, 1:2], in1=tmp1,
                    op=OP.add)
            # group sums across partitions: [G, 2] = sum_ci ag[ci]^T stats2[ci]
            pg = ps_t.tile([G, 2], f32, tag="t")
            for ci in range(2):
                nc.tensor.matmul(pg, lhsT=ag[:, ci, :], rhs=stats2[:, ci, :],
                                 start=(ci == 0), stop=(ci == 1))
            # ag carries 1/CPG so pg is directly (mean_g, E[x^2]_g)
            pgs = gnp.tile([G, 2], f32, tag="pgs")
            nc.vector.tensor_copy(out=pgs, in_=pg)
            gst = gnp.tile([G, 4], f32, tag="gst")  # mean^2, var, sd, -
            nc.vector.tensor_tensor(out=gst[:, 0:1], in0=pgs[:, 0:1],
                                    in1=pgs[:, 0:1], op=OP.mult)
            nc.vector.tensor_tensor(out=gst[:, 1:2], in0=pgs[:, 1:2],
                                    in1=gst[:, 0:1], op=OP.subtract)
            gfin = gnp.tile([G, 2], f32, tag="gfin")  # (rstd_g, mean_g*rstd_g)
            nc.scalar.activation(out=gst[:, 2:3], in_=gst[:, 1:2],
                                 func=AF.Sqrt, bias=epst)
            nc.vector.reciprocal(out=gfin[:, 0:1], in_=gst[:, 2:3])
            nc.vector.tensor_tensor(out=gfin[:, 1:2], in0=pgs[:, 0:1],
                                    in1=gfin[:, 0:1], op=OP.mult)
            # bg carries gn_w, so pbc = (scale_c, mean_c*scale_c);
            # bias_c = gn_b - mean_c*scale_c
            scbc = gnp.tile([128, 2, 2], f32, tag="scbc")
            for ci in range(2):
                pbc = ps_t.tile([128, 2], f32, tag="t")
                nc.tensor.matmul(pbc, lhsT=bg[:, ci, :], rhs=gfin,
                                 start=True, stop=True)
                nc.vector.tensor_copy(out=scbc[:, ci, 0:1], in_=pbc[:, 0:1])
                nc.vector.tensor_tensor(out=scbc[:, ci, 1:2], in0=gb[:, ci, :],
                                        in1=pbc[:, 1:2], op=OP.subtract)

            # ---- fold GN into conv weights: W' = W .* scale_c, cast fp8.
            # k section first (kt conv gates the attention loop).
            wall8 = cst.tile([128, 2, 3 * C], fp8, tag="wall8")
            for sec in (1, 0, 2):                 # k, q, v
                for ci in range(2):
                    nc.vector.tensor_scalar(
                        out=wall8[:, ci, sec * C:(sec + 1) * C],
                        in0=wall[:, ci, sec * C:(sec + 1) * C],
                        scalar1=scbc[:, ci, 0:1], scalar2=None, op0=OP.mult)
            w8q, w8k, w8v = (wall8[:, :, i * C:(i + 1) * C] for i in range(3))
            wqs, wks, wvs = (wall[:, :, i * C:(i + 1) * C] for i in range(3))

            # effective channel biases (tiny f32r matmuls on unscaled W):
            # bias_c duplicated to 2 cols for an even f32r moving dim
            bcc = cst.tile([128, 2, 2], f32r, tag="bcc")
            for ci in range(2):
                nc.vector.tensor_copy(out=bcc[:, ci, 0:1], in_=scbc[:, ci, 1:2])
                nc.vector.tensor_copy(out=bcc[:, ci, 1:2], in_=scbc[:, ci, 1:2])
            # qb2/kb2 = b + W^T bias_c   (cols of bias2: [q, k])
            bias2 = gnp.tile([128, 2, 2], f32, tag="bias2")
            for wi, wsl in enumerate((wqs, wks)):
                for cb in range(2):
                    pbias = ps_t.tile([128, 2], f32, tag="t")
                    for ci in range(2):
                        nc.tensor.matmul(
                            pbias,
                            lhsT=wsl[:, ci, cb * 128:(cb + 1) * 128],
                            rhs=bcc[:, ci, :], start=(ci == 0), stop=(ci == 1))
                    nc.vector.tensor_tensor(
                        out=bias2[:, cb, wi:wi + 1], in0=pbias[:, 0:1],
                        in1=(qb if wi == 0 else kb)[:, cb, :], op=OP.add)
            # v bias along FREE dim: vb2[1, c_out] = bias_c^T Wv + vb, then
            # partition-broadcast via a K=1 f32r matmul
            pvb = ps_t.tile([1, 512], f32, tag="t")
            for ci in range(2):
                nc.tensor.matmul(pvb[:, 0:C], lhsT=bcc[:, ci, 0:1],
                                 rhs=wvs[:, ci, :], start=(ci == 0),
                                 stop=(ci == 1))
            vb2r = gnp.tile([1, C], f32r, tag="vb2r")
            nc.vector.tensor_copy(out=vb2r, in_=pvb[:, 0:C])
            vb2b = gnp.tile([128, C], f32, tag="vb2b")
            pvbb = ps_t.tile([128, 512], f32, tag="t")
            nc.tensor.matmul(pvbb[:, 0:C], lhsT=ones_row, rhs=vb2r,
                             start=True, stop=True)
            nc.vector.tensor_copy(out=vb2b, in_=pvbb[:, 0:C])

            nc.scalar.activation(out=warm[:, 0:1], in_=warm[:, 0:1],
                                 func=AF.Exp)

            # ---- qkv convs (fp8 DoubleRow; K=256 in one matmul) ----
            # Each section rotates matmul outputs through the slots of one
            # 4-bank PSUM tile so independent matmuls pipeline on the PE
            # (216ns/512-row issue pace) while DVE epilogues trail behind.
            # q first, then kt in j-major order so attention unblocks early.
            qt = big.tile([128, 2, 4, 512], fp8, tag="qt")
            sq = ps_s.tile([128, 4, 512], f32, tag="s")
            n = 0
            for t in range(2):
                for cb in range(2):
                    s0 = (2 * n) % 4
                    for i in range(2):
                        j = 2 * t + i
                        nc.tensor.matmul(
                            sq[:, s0 + i, :],
                            lhsT=w8q[:, :, cb * 128:(cb + 1) * 128],
                            rhs=x8h[:, :, j * 512:(j + 1) * 512],
                            start=True, stop=True, perf_mode=DR)
                    nc.vector.tensor_scalar(
                        out=qt[:, cb, 2 * t:2 * t + 2, :],
                        in0=sq[:, s0:s0 + 2, :],
                        scalar1=bias2[:, cb, 0:1], scalar2=None, op0=OP.add)
                    n += 1
            kt = big.tile([128, 2, 8, 512], fp8, tag="kt")
            sk = ps_s.tile([128, 4, 512], f32, tag="s")
            n = 0
            for j2 in range(4):
                for cb in range(2):
                    s0 = (2 * n) % 4
                    for i in range(2):
                        j = 2 * j2 + i
                        nc.tensor.matmul(
                            sk[:, s0 + i, :],
                            lhsT=w8k[:, :, cb * 128:(cb + 1) * 128],
                            rhs=x8[:, :, j * 512:(j + 1) * 512],
                            start=True, stop=True, perf_mode=DR)
                    nc.vector.tensor_scalar(
                        out=kt[:, cb, 2 * j2:2 * j2 + 2, :],
                        in0=sk[:, s0:s0 + 2, :],
                        scalar1=bias2[:, cb, 1:2], scalar2=None, op0=OP.add)
                    n += 1
            # vb2b duplicated into the pair layout for one-op epilogues
            vb22 = gnp.tile([128, 2, C], f32, tag="vb22")
            nc.vector.tensor_copy(out=vb22[:, 0, :], in_=vb2b)
            nc.vector.tensor_copy(out=vb22[:, 1, :], in_=vb2b)
            vT = big.tile([128, MB, C], fp8, tag="vT")
            sv = ps_s.tile([128, 4, 512], f32, tag="s")
            for p in range(NP):
                s0 = (2 * p) % 4
                for i in range(2):
                    nc.tensor.matmul(
                        sv[:, s0 + i, 0:C],
                        lhsT=x8[:, :, (2 * p + i) * 128:(2 * p + i + 1) * 128],
                        rhs=wall8[:, :, 2 * C:3 * C],
                        start=True, stop=True, perf_mode=DR)
                nc.vector.tensor_tensor(
                    out=vT[:, 2 * p:2 * p + 2, :], in0=sv[:, s0:s0 + 2, 0:C],
                    in1=vb22, op=OP.add)

            # residual-with-bias, needed only at chunk tails (emitted late so
            # it does not delay bn_stats in the DVE queue)
            xo = big.tile([128, 2, NH], f32, tag="xo")  # x_half + rbias
            for ci in range(2):
                for j in range(2):
                    sl = slice(j * 1024, (j + 1) * 1024)
                    nc.vector.tensor_scalar(
                        out=xo[:, ci, sl], in0=xh[:, ci, sl],
                        scalar1=rbias[:, ci, :], scalar2=None, op0=OP.add)

            # ---- attention (software-pipelined: the PE runs 2 key-block
            # pairs ahead on S while ACT exps the current pair, so the
            # in-order PE queue never waits out the exp latency) ----
            for j in range(NH // 512):
                sl = slice(j * 512, (j + 1) * 512)
                po = ps_o.tile([128, 3, 512], f32, tag="o")  # O c0, O c1, R
                sb4 = ps_s.tile([128, 4, 512], f32, tag="s")

                def s_pair(p):
                    for i in range(2):
                        mb = 2 * p + i
                        nc.tensor.matmul(
                            sb4[:, (2 * p + i) % 4, :],
                            lhsT=kt[:, :, mb // 4, (mb % 4) * 128:(mb % 4 + 1) * 128],
                            rhs=qt[:, :, j, :], start=True, stop=True,
                            perf_mode=DR)

                s_pair(0)
                s_pair(1)
                for p in range(NP):
                    s0 = (2 * p) % 4
                    et = epool.tile([128, 2, 512], fp8, tag="et")
                    nc.scalar.activation(out=et, in_=sb4[:, s0:s0 + 2, :],
                                         func=AF.Exp, scale=1.0 / 16.0,
                                         bias=negone)
                    if p + 2 < NP:
                        s_pair(p + 2)
                    for cb in range(2):
                        nc.tensor.matmul(
                            po[:, cb, :],
                            lhsT=vT[:, 2 * p:2 * p + 2, cb * 128:(cb + 1) * 128],
                            rhs=et, start=(p == 0), stop=(p == NP - 1),
                            perf_mode=DR, skip_group_check=True)
                    nc.tensor.matmul(
                        po[0:1, 2, :], lhsT=ones21, rhs=et,
                        start=(p == 0), stop=(p == NP - 1),
                        perf_mode=DR, skip_group_check=True)
                # Free po fast: copy R and both O banks out immediately;
                # the reciprocal then runs on the SBUF copy without holding po.
                rsb = wrk.tile([1, 512], f32, tag="rsb")
                nc.vector.tensor_copy(out=rsb, in_=po[0:1, 2, :])
                onorm = wrk.tile([128, 2, 512], fp8, tag="onorm")
                nc.vector.tensor_scalar(
                    out=onorm[:, 0, :], in0=po[:, 0, :], scalar1=inv64,
                    scalar2=None, op0=OP.mult)
                nc.vector.tensor_scalar(
                    out=onorm[:, 1, :], in0=po[:, 1, :], scalar1=inv64,
                    scalar2=None, op0=OP.mult)
                rinvf = wrk.tile([1, 512], f32, tag="rinvf")
                nc.vector.reciprocal_approx_fast(out=rinvf, in_=rsb)
                rinv = wrk.tile([1, 512], f32r, tag="rinv")
                nc.vector.tensor_copy(out=rinv, in_=rinvf)
                last = (j == NH // 512 - 1)
                if last:
                    # final chunk: keep PE's last matmuls off the slow
                    # reciprocal chain -- proj first (into ps_t + po's freed
                    # O bank), broadcast goes to po's freed R bank.
                    pps = []
                    for cb in range(2):
                        if cb == 0:
                            pp = ps_t.tile([128, 512], f32, tag="t",
                                           name="pp_last")
                        else:
                            pp = po[:, 1, :]
                        nc.tensor.matmul(
                            pp, lhsT=wp8[:, :, cb * 128:(cb + 1) * 128],
                            rhs=onorm, start=True, stop=True, perf_mode=DR,
                            skip_group_check=True)
                        pps.append(pp)
                    nc.tensor.matmul(po[:, 2, :], lhsT=row64, rhs=rinv,
                                     start=True, stop=True,
                                     skip_group_check=True)
                    rb = wrk.tile([128, 512], f32, tag="rb")
                    nc.vector.tensor_copy(out=rb, in_=po[:, 2, :])
                    for cb in range(2):
                        outt = wrk.tile([128, 512], f32, tag="outt")
                        nc.vector.tensor_tensor(out=outt, in0=pps[cb], in1=rb,
                                                op=OP.mult)
                        nc.vector.tensor_tensor(out=outt, in0=outt,
                                                in1=xo[:, cb, sl], op=OP.add)
                        nc.sync.dma_start(out=d_out.ap()[:, cb, sl], in_=outt)
                else:
                    pbx = ps_t.tile([128, 512], f32, tag="t")
                    nc.tensor.matmul(pbx, lhsT=row64, rhs=rinv,
                                     start=True, stop=True)
                    rb = wrk.tile([128, 512], f32, tag="rb")
                    nc.vector.tensor_copy(out=rb, in_=pbx)
                    for cb in range(2):
                        pp = ps_t.tile([128, 512], f32, tag="t")
                        nc.tensor.matmul(
                            pp, lhsT=wp8[:, :, cb * 128:(cb + 1) * 128],
                            rhs=onorm, start=True, stop=True, perf_mode=DR)
                        outt = wrk.tile([128, 512], f32, tag="outt")
                        nc.vector.tensor_tensor(out=outt, in0=pp, in1=rb,
                                                op=OP.mult)
                        nc.vector.tensor_tensor(out=outt, in0=outt,
                                                in1=xo[:, cb, sl], op=OP.add)
                        nc.sync.dma_start(out=d_out.ap()[:, cb, sl], in_=outt)

    nc.compile()
    _cache["nc"] = nc
    return nc


def _prep_maps(x, gn_w, gn_b, qkv_w, qkv_b, proj_w, proj_b):
    """Host-side sharding + layout prep. Returns list of 8 in_maps."""
    import ml_dtypes
    fp8 = ml_dtypes.float8_e4m3
    x = np.asarray(x, np.float32)
    qkv_w = np.asarray(qkv_w, np.float32)
    qkv_b = np.asarray(qkv_b, np.float32)
    proj_w = np.asarray(proj_w, np.float32)
    proj_b = np.asarray(proj_b, np.float32)
    gn_w = np.asarray(gn_w, np.float32)
    gn_b = np.asarray(gn_b, np.float32)

    def chunked(a):  # [256, ...] -> [128, 2, ...]
        return np.ascontiguousarray(a.reshape(2, 128, *a.shape[1:]).transpose(
            1, 0, *range(2, a.ndim + 1)))

    wq = chunked(qkv_w[0:C].T.copy())          # [c_in, c_out] -> [128,2,C]
    wk = chunked(qkv_w[C:2 * C].T.copy())
    wv = chunked(qkv_w[2 * C:3 * C].T.copy())
    wp8 = chunked(proj_w.T.copy()).astype(fp8)
    rbias = proj_w @ qkv_b[2 * C:3 * C] + proj_b   # v-bias fold + proj bias
    smalls = np.stack([qkv_b[0:C], qkv_b[C:2 * C], gn_w, gn_b, rbias], axis=1)
    smalls = chunked(smalls)

    cidx = np.arange(C)
    ag_full = (cidx[:, None] // CPG == np.arange(G)[None, :]).astype(np.float32)
    ag = chunked(ag_full / CPG)                     # [128, 2, G], carries 1/8
    bg_full = ag_full * gn_w[:, None]               # carries gn_w
    bg = np.ascontiguousarray(
        bg_full.reshape(2, 128, G).transpose(2, 0, 1))  # [G, 2, 128]

    maps = []
    for core in range(8):
        b, half = core // 2, core % 2
        xf = x[b].reshape(C, HW)
        xh = xf[:, half * NH:(half + 1) * NH]
        maps.append({
            "x8": chunked(xf).astype(fp8),
            "x8h": chunked(xh).astype(fp8), "xh": chunked(xh),
            "wq": wq, "wk": wk, "wv": wv, "wp8": wp8,
            "sb": smalls, "ag": ag, "bg": bg,
        })
    return maps


def kernel(x, gn_w, gn_b, qkv_w, qkv_b, proj_w, proj_b):
    import concourse.bass_utils as bu
    nc = build_nc()
    maps = _prep_maps(x, gn_w, gn_b, qkv_w, qkv_b, proj_w, proj_b)
    res = bu.run_bass_kernel_spmd(nc, maps, core_ids=list(range(8)))
    out = np.empty((B, C, HW), np.float32)
    for core in range(8):
        b, half = core // 2, core % 2
        o = res.results[core]["out"]                # [128, 2, NH]
        out[b, :, half * NH:(half + 1) * NH] = \
            o.transpose(1, 0, 2).reshape(C, NH)
    return out.reshape(B, C, 64, 64)


# revision 13
# speedup vs baseline: 1.6030x; 1.6030x over previous
"""AttentionBlock (B=4, C=256, H=W=64) on 8 Trainium2 NeuronCores.

Sharding: data-parallel over (batch, query-half): core i handles batch i//2,
query pixels [half*2048, (half+1)*2048), half = i%2. GroupNorm stats are
computed per batch element (duplicated across the pair, cheap); the O(N^2)
attention work is fully sharded 8 ways. No collectives.

v4: fp8e4 DoubleRow matmuls (K=256 per instruction) + algebraic fusion of
the k/v 1x1 convs into the attention matmuls, and software-pipelined
emission so the in-order PE queue never waits out the exp latency.

  S[m,n]  = sum_c x[c,m] * qq[c,n],   qq = s .* (Wk^T q)   (k-conv fused)
  O[o,n]  = sum_c Wv[c,o] * s[c] * xe[c,n],  xe = sum_m x[c,m] E[m,n]
                                              (v-conv fused, post-E)
  v-bias term vb2 = Wv^T bias_c surfaces as + (Wp^T vb2) in the output and
  is folded into the residual bias (rank-1 in R, cancels against 1/R).

Per 512-query chunk: 16 key-block PAIRS; per pair g the emission order is
  exp(g) [ACT] ; xe(g-1), R(g-1) [PE] ; S-pair(g+1) [PE]
so during exp(g) the PE executes {S(g+1), xe(g-1), R(g-1)} whose deps are
all <= exp(g-1): steady state ~1.15us/pair on both engines.
E = exp(S/16 - 2.5) in fp8 (shift cancels in softmax, keeps E < fp8e4 max);
xe is accumulated /64-scaled into fp8 (xe can exceed fp8 range), the x64
restored via the R-broadcast (row of 64s). Softmax denominators R come from
a DoubleRow ones-matmul accumulated alongside xe; 1/R via the fast DVE
reciprocal approximation (~18 bits, ample)."""

import numpy as np

B, C, HW = 4, 256, 4096
NH = 2048            # query pixels per core
G, CPG = 32, 8       # groups, channels per group
EPS = 1e-5
MB = HW // 128       # 32 key blocks
NP = MB // 2         # 16 key-block pairs

_cache = {}


def build_nc():
    """Build (and cache) the Bass module."""
    if "nc" in _cache:
        return _cache["nc"]
    import concourse.tile as tile
    from concourse import bacc, mybir

    f32 = mybir.dt.float32
    f32r = mybir.dt.float32r
    fp8 = mybir.dt.float8e4
    bf16 = mybir.dt.bfloat16
    AF = mybir.ActivationFunctionType
    OP = mybir.AluOpType
    DR = mybir.MatmulPerfMode.DoubleRow

    nc = bacc.Bacc("TRN2", target_bir_lowering=False, debug=False,
                   enable_asserts=False, num_devices=8)

    # ---- DRAM I/O (host preps everything into device layout) ----
    d_x8 = nc.dram_tensor("x8", [128, 2, HW], fp8, kind="ExternalInput")
    d_x8h = nc.dram_tensor("x8h", [128, 2, NH], fp8, kind="ExternalInput")
    d_xT8 = nc.dram_tensor("xT8", [128, MB, C], fp8, kind="ExternalInput")
    d_xh = nc.dram_tensor("xh", [128, 2, NH], f32, kind="ExternalInput")
    d_wq = nc.dram_tensor("wq", [128, 2, C], f32r, kind="ExternalInput")
    d_wvf = nc.dram_tensor("wvf", [128, 2, C], f32r, kind="ExternalInput")
    d_wpf = nc.dram_tensor("wpf", [128, 2, C], f32r, kind="ExternalInput")
    d_wkTb = nc.dram_tensor("wkTb", [128, 2, C], bf16, kind="ExternalInput")
    d_wv8 = nc.dram_tensor("wv8", [128, 2, C], fp8, kind="ExternalInput")
    d_wp8 = nc.dram_tensor("wp8", [128, 2, C], fp8, kind="ExternalInput")
    d_sb = nc.dram_tensor("sb", [128, 2, 5], f32, kind="ExternalInput")
    d_ag = nc.dram_tensor("ag", [128, 2, G], f32, kind="ExternalInput")
    d_bg = nc.dram_tensor("bg", [G, 2, 128], f32, kind="ExternalInput")
    d_out = nc.dram_tensor("out", [128, 2, NH], f32, kind="ExternalOutput")

    with tile.TileContext(nc) as tc:
        with (
            tc.tile_pool(name="big", bufs=1) as big,
            tc.tile_pool(name="cst", bufs=1) as cst,
            tc.tile_pool(name="wrk", bufs=2) as wrk,
            tc.tile_pool(name="epool", bufs=4) as epool,
            tc.tile_pool(name="gnp", bufs=1) as gnp,
            tc.tile_pool(name="ps_s", bufs=2, space="PSUM") as ps_s,
            tc.tile_pool(name="ps_o", bufs=1, space="PSUM") as ps_o,
            tc.tile_pool(name="ps_t", bufs=1, space="PSUM") as ps_t,
        ):
            # ---- ACT table warm: sqrt first (GN needs it soon); exp is
            # re-warmed after the GN chain, ahead of the attention stream.
            warm = cst.tile([1, 2], f32, tag="warm")
            nc.vector.memset(warm, 1.0)
            nc.scalar.activation(out=warm[:, 1:2], in_=warm[:, 1:2],
                                 func=AF.Sqrt)
            smalls = cst.tile([128, 2, 5], f32, tag="smalls")
            nc.scalar.dma_start(out=smalls, in_=d_sb.ap())
            qb = smalls[:, :, 0:1]
            gb = smalls[:, :, 3:4]
            rbias = smalls[:, :, 4:5]
            ag = cst.tile([128, 2, G], f32, tag="ag")
            nc.scalar.dma_start(out=ag, in_=d_ag.ap())
            bg = cst.tile([G, 2, 128], f32, tag="bg")
            nc.scalar.dma_start(out=bg, in_=d_bg.ap())

            # ---- input loads (x8 split across the SP and Pool DGE rings so
            # bn_stats can start ~6us in; weights go on the ACT ring) ----
            x8 = big.tile([128, 2, HW], fp8, tag="x8")
            for ci in range(2):
                for j in range(8):
                    sl = slice(j * 512, (j + 1) * 512)
                    eng = nc.sync if (j % 2 == 0) else nc.gpsimd
                    eng.dma_start(out=x8[:, ci, sl], in_=d_x8.ap()[:, ci, sl])
            x8h = big.tile([128, 2, NH], fp8, tag="x8h")
            for ci in range(2):
                nc.gpsimd.dma_start(out=x8h[:, ci, :], in_=d_x8h.ap()[:, ci, :])
            xT8 = big.tile([128, MB, C], fp8, tag="xT8")
            for q4 in range(4):
                nc.gpsimd.dma_start(out=xT8[:, q4 * 8:(q4 + 1) * 8, :],
                                    in_=d_xT8.ap()[:, q4 * 8:(q4 + 1) * 8, :])
            wall = cst.tile([128, 2, 3 * C], f32r, tag="wall")
            for i, d in enumerate((d_wq, d_wvf, d_wpf)):
                nc.scalar.dma_start(out=wall[:, :, i * C:(i + 1) * C], in_=d.ap())
            wqs = wall[:, :, 0:C]
            wvf = wall[:, :, C:2 * C]
            wpf = wall[:, :, 2 * C:3 * C]
            wkTb = cst.tile([128, 2, C], bf16, tag="wkTb")
            nc.scalar.dma_start(out=wkTb, in_=d_wkTb.ap())
            wv8 = cst.tile([128, 2, C], fp8, tag="wv8")
            nc.scalar.dma_start(out=wv8, in_=d_wv8.ap())
            wp8 = cst.tile([128, 2, C], fp8, tag="wp8")
            nc.scalar.dma_start(out=wp8, in_=d_wp8.ap())
            xh = big.tile([128, 2, NH], f32, tag="xh")
            for ci in range(2):
                for j in range(2):
                    sl = slice(j * 1024, (j + 1) * 1024)
                    nc.sync.dma_start(out=xh[:, ci, sl], in_=d_xh.ap()[:, ci, sl])

            # ---- GroupNorm stats (fp8 x; quantization noise ~1e-3 rel) ----
            bstat = gnp.tile([128, 2, 8, 6], f32, tag="bstat")
            for ci in range(2):
                for j in range(8):
                    nc.vector.bn_stats(
                        out=bstat[:, ci, j, :],
                        in_=x8[:, ci, j * 512:(j + 1) * 512])

            # constants (DVE, after bn_stats in the queue: all tiny)
            epst = cst.tile([G, 1], f32, tag="epst")
            nc.vector.memset(epst, EPS)
            ones21t = cst.tile([128, 2, 16], fp8, tag="ones21")
            nc.vector.memset(ones21t, 1.0)
            ones21 = ones21t[:, :, 0:1]    # R lhsT (DR, 16-aligned pair dim)
            negc = cst.tile([128, 1], f32, tag="negc")  # exp shift
            nc.vector.memset(negc, -2.5)
            onesr = cst.tile([1, 128], f32, tag="onesr")
            nc.vector.memset(onesr, 1.0)
            ones_row = cst.tile([1, 128], f32r, tag="ones_row")
            nc.vector.tensor_copy(out=ones_row, in_=onesr)
            r64s = cst.tile([1, 128], f32, tag="r64s")
            nc.vector.memset(r64s, 64.0)
            row64 = cst.tile([1, 128], f32r, tag="row64")   # 64/R bcast lhsT
            nc.vector.tensor_copy(out=row64, in_=r64s)
            inv64 = cst.tile([128, 1], f32, tag="inv64")
            nc.vector.memset(inv64, 1.0 / 64.0)

            stats2 = gnp.tile([128, 2, 2], f32, tag="stats2")  # (mean, E[x^2])
            tmp1 = gnp.tile([128, 1], f32, tag="tmp1")
            for ci in range(2):
                nc.vector.bn_aggr(out=stats2[:, ci, :], in_=bstat[:, ci, :, :])
                nc.vector.tensor_tensor(
                    out=tmp1, in0=stats2[:, ci, 0:1], in1=stats2[:, ci, 0:1],
                    op=OP.mult)
                nc.vector.tensor_tensor(
                    out=stats2[:, ci, 1:2], in0=stats2[:, ci, 1:2], in1=tmp1,
                    op=OP.add)
            pg = ps_t.tile([G, 2], f32, tag="t")
            for ci in range(2):
                nc.tensor.matmul(pg, lhsT=ag[:, ci, :], rhs=stats2[:, ci, :],
                                 start=(ci == 0), stop=(ci == 1))
            pgs = gnp.tile([G, 2], f32, tag="pgs")
            nc.vector.tensor_copy(out=pgs, in_=pg)
            gst = gnp.tile([G, 4], f32, tag="gst")
            nc.vector.tensor_tensor(out=gst[:, 0:1], in0=pgs[:, 0:1],
                                    in1=pgs[:, 0:1], op=OP.mult)
            nc.vector.tensor_tensor(out=gst[:, 1:2], in0=pgs[:, 1:2],
                                    in1=gst[:, 0:1], op=OP.subtract)
            gfin = gnp.tile([G, 2], f32, tag="gfin")  # (rstd_g, mean_g*rstd_g)
            nc.scalar.activation(out=gst[:, 2:3], in_=gst[:, 1:2],
                                 func=AF.Sqrt, bias=epst)
            nc.vector.reciprocal(out=gfin[:, 0:1], in_=gst[:, 2:3])
            nc.vector.tensor_tensor(out=gfin[:, 1:2], in0=pgs[:, 0:1],
                                    in1=gfin[:, 0:1], op=OP.mult)
            # bg carries gn_w: pbc = (scale_c, mean_c*scale_c);
            # bias_c = gn_b - mean_c*scale_c
            scbc = gnp.tile([128, 2, 2], f32, tag="scbc")
            for ci in range(2):
                pbc = ps_t.tile([128, 2], f32, tag="t")
                nc.tensor.matmul(pbc, lhsT=bg[:, ci, :], rhs=gfin,
                                 start=True, stop=True)
                nc.vector.tensor_copy(out=scbc[:, ci, 0:1], in_=pbc[:, 0:1])
                nc.vector.tensor_tensor(out=scbc[:, ci, 1:2], in0=gb[:, ci, :],
                                        in1=pbc[:, 1:2], op=OP.subtract)

            # q weights: fold GN scale, cast fp8 (q-conv is the only conv)
            w8q = cst.tile([128, 2, C], fp8, tag="w8q")
            for ci in range(2):
                nc.vector.tensor_scalar(
                    out=w8q[:, ci, :], in0=wqs[:, ci, :],
                    scalar1=scbc[:, ci, 0:1], scalar2=None, op0=OP.mult)

            # bias chain (tiny f32r matmuls):
            #   bias2q = qb + Wq^T bias_c         (per q out-channel)
            #   vb2    = Wv^T bias_c              (partition layout)
            #   ub     = Wp^T vb2                 -> rbias2 = rbias + ub
            bcc = cst.tile([128, 2, 2], f32r, tag="bcc")
            for ci in range(2):
                nc.vector.tensor_copy(out=bcc[:, ci, 0:1], in_=scbc[:, ci, 1:2])
                nc.vector.tensor_copy(out=bcc[:, ci, 1:2], in_=scbc[:, ci, 1:2])
            bias2q = gnp.tile([128, 2, 1], f32, tag="bias2q")
            for cb in range(2):
                pbias = ps_t.tile([128, 2], f32, tag="t")
                for ci in range(2):
                    nc.tensor.matmul(
                        pbias, lhsT=wqs[:, ci, cb * 128:(cb + 1) * 128],
                        rhs=bcc[:, ci, :], start=(ci == 0), stop=(ci == 1))
                nc.vector.tensor_tensor(
                    out=bias2q[:, cb, :], in0=pbias[:, 0:1],
                    in1=qb[:, cb, :], op=OP.add)
            vb2pr = gnp.tile([128, 2, 2], f32r, tag="vb2pr")
            for cb in range(2):
                pvb = ps_t.tile([128, 2], f32, tag="t")
                for ci in range(2):
                    nc.tensor.matmul(
                        pvb, lhsT=wvf[:, ci, cb * 128:(cb + 1) * 128],
                        rhs=bcc[:, ci, :], start=(ci == 0), stop=(ci == 1))
                nc.vector.tensor_copy(out=vb2pr[:, cb, :], in_=pvb)
            rbias2 = gnp.tile([128, 2, 1], f32, tag="rbias2")
            for cb in range(2):
                pub = ps_t.tile([128, 2], f32, tag="t")
                for ch in range(2):
                    nc.tensor.matmul(
                        pub, lhsT=wpf[:, ch, cb * 128:(cb + 1) * 128],
                        rhs=vb2pr[:, ch, :], start=(ch == 0), stop=(ch == 1))
                nc.vector.tensor_tensor(
                    out=rbias2[:, cb, :], in0=pub[:, 0:1],
                    in1=rbias[:, cb, :], op=OP.add)

            nc.scalar.activation(out=warm[:, 0:1], in_=warm[:, 0:1],
                                 func=AF.Exp)

            # ---- q conv (fp8 DR) -> qt bf16; epilogues on ACT ----
            qt = big.tile([128, 2, 4, 512], bf16, tag="qt")
            qtiles = []
            for n in range(4):
                cb, t = n % 2, n // 2
                pq = ps_s.tile([128, 2, 512], f32, tag="s")
                for i in range(2):
                    j = 2 * t + i
                    nc.tensor.matmul(
                        pq[:, i, :], lhsT=w8q[:, :, cb * 128:(cb + 1) * 128],
                        rhs=x8h[:, :, j * 512:(j + 1) * 512],
                        start=True, stop=True, perf_mode=DR)
                qtiles.append((pq, cb, t))
                if n >= 1:
                    opq, ocb, ot = qtiles[n - 1]
                    nc.scalar.activation(
                        out=qt[:, ocb, 2 * ot:2 * ot + 2, :], in_=opq,
                        func=AF.Identity, bias=bias2q[:, ocb, :])
            opq, ocb, ot = qtiles[3]
            nc.scalar.activation(out=qt[:, ocb, 2 * ot:2 * ot + 2, :],
                                 in_=opq, func=AF.Identity,
                                 bias=bias2q[:, ocb, :])

            # ---- qq = s .* (Wk^T q)  (bf16 matmuls; epilogues on DVE) ----
            qq8 = big.tile([128, 2, 4, 512], fp8, tag="qq8")
            qqtiles = []
            for j in range(4):
                pqq = ps_s.tile([128, 2, 512], f32, tag="s")
                for ci in range(2):
                    for ch in range(2):
                        nc.tensor.matmul(
                            pqq[:, ci, :],
                            lhsT=wkTb[:, ch, ci * 128:(ci + 1) * 128],
                            rhs=qt[:, ch, j, :], start=(ch == 0),
                            stop=(ch == 1))
                qqtiles.append(pqq)
                if j >= 1:
                    for ci in range(2):
                        nc.vector.tensor_scalar(
                            out=qq8[:, ci, j - 1, :],
                            in0=qqtiles[j - 1][:, ci, :],
                            scalar1=scbc[:, ci, 0:1], scalar2=None,
                            op0=OP.mult)
            for ci in range(2):
                nc.vector.tensor_scalar(
                    out=qq8[:, ci, 3, :], in0=qqtiles[3][:, ci, :],
                    scalar1=scbc[:, ci, 0:1], scalar2=None, op0=OP.mult)

            # residual-with-bias (late in the DVE queue; needed at tails)
            xo = big.tile([128, 2, NH], f32, tag="xo")
            for cb in range(2):
                for j in range(2):
                    sl = slice(j * 1024, (j + 1) * 1024)
                    nc.vector.tensor_scalar(
                        out=xo[:, cb, sl], in0=xh[:, cb, sl],
                        scalar1=rbias2[:, cb, :], scalar2=None, op0=OP.add)

            # ---- attention ----
            for j in range(NH // 512):
                sl = slice(j * 512, (j + 1) * 512)
                po = ps_o.tile([128, 3, 512], f32, tag="o")  # xe c0, xe c1, R

                stiles = [None] * NP

                def s_pair(g):
                    st = ps_s.tile([128, 2, 512], f32, tag="s")
                    stiles[g] = st
                    for i in range(2):
                        mb = 2 * g + i
                        nc.tensor.matmul(
                            st[:, i, :],
                            lhsT=x8[:, :, mb * 128:(mb + 1) * 128],
                            rhs=qq8[:, :, j, :], start=True, stop=True,
                            perf_mode=DR)

                def xe_r(g, et):
                    for cb in range(2):
                        nc.tensor.matmul(
                            po[:, cb, :],
                            lhsT=xT8[:, 2 * g:2 * g + 2, cb * 128:(cb + 1) * 128],
                            rhs=et, start=(g == 0), stop=(g == NP - 1),
                            perf_mode=DR, skip_group_check=True)
                    nc.tensor.matmul(
                        po[0:1, 2, :], lhsT=ones21, rhs=et,
                        start=(g == 0), stop=(g == NP - 1),
                        perf_mode=DR, skip_group_check=True)

                s_pair(0)
                s_pair(1)
                ets = [None] * NP
                for g in range(NP):
                    et = epool.tile([128, 2, 512], fp8, tag="et")
                    ets[g] = et
                    nc.scalar.activation(out=et, in_=stiles[g], func=AF.Exp,
                                         scale=1.0 / 16.0, bias=negc)
                    if g >= 1:
                        xe_r(g - 1, ets[g - 1])
                    if g + 2 <= NP - 1:
                        s_pair(g + 2)
                xe_r(NP - 1, ets[NP - 1])

                # tail: xe -> fp8 (x GN-scale x 1/64) frees po early; then
                # O = Wv^T xe8 into po's freed banks; proj via ps_t with the
                # 64/R broadcast; out = proj*(64/R) + xo.
                rsb = wrk.tile([1, 512], f32, tag="rsb")
                nc.vector.tensor_copy(out=rsb, in_=po[0:1, 2, :])
                xe8 = wrk.tile([128, 2, 512], fp8, tag="xe8")
                for cb in range(2):
                    nc.vector.tensor_scalar(
                        out=xe8[:, cb, :], in0=po[:, cb, :],
                        scalar1=scbc[:, cb, 0:1], scalar2=inv64,
                        op0=OP.mult, op1=OP.mult)
                rinvf = wrk.tile([1, 512], f32, tag="rinvf")
                nc.vector.reciprocal_approx_fast(out=rinvf, in_=rsb)
                rinv = wrk.tile([1, 512], f32r, tag="rinv")
                nc.vector.tensor_copy(out=rinv, in_=rinvf)
                for co in range(2):
                    nc.tensor.matmul(
                        po[:, co, :], lhsT=wv8[:, :, co * 128:(co + 1) * 128],
                        rhs=xe8, start=True, stop=True, perf_mode=DR,
                        skip_group_check=True)
                onorm = wrk.tile([128, 2, 512], fp8, tag="onorm")
                for co in range(2):
                    nc.vector.tensor_copy(out=onorm[:, co, :], in_=po[:, co, :])
                pbx = ps_t.tile([128, 512], f32, tag="t")
                nc.tensor.matmul(pbx, lhsT=row64, rhs=rinv,
                                 start=True, stop=True)
                rb = wrk.tile([128, 512], f32, tag="rb")
                nc.vector.tensor_copy(out=rb, in_=pbx)
                for co in range(2):
                    pp = ps_t.tile([128, 512], f32, tag="t")
                    nc.tensor.matmul(
                        pp, lhsT=wp8[:, :, co * 128:(co + 1) * 128],
                        rhs=onorm, start=True, stop=True, perf_mode=DR)
                    outt = wrk.tile([128, 512], f32, tag="outt")
                    nc.vector.tensor_tensor(out=outt, in0=pp, in1=rb,
                                            op=OP.mult)
                    nc.vector.tensor_tensor(out=outt, in0=outt,
                                            in1=xo[:, co, sl], op=OP.add)
                    nc.sync.dma_start(out=d_out.ap()[:, co, sl], in_=outt)

    nc.compile()
    _cache["nc"] = nc
    return nc


def _prep_maps(x, gn_w, gn_b, qkv_w, qkv_b, proj_w, proj_b):
    """Host-side sharding + layout prep. Returns list of 8 in_maps."""
    import ml_dtypes
    fp8 = ml_dtypes.float8_e4m3
    bf16 = ml_dtypes.bfloat16
    x = np.asarray(x, np.float32)
    qkv_w = np.asarray(qkv_w, np.float32)
    qkv_b = np.asarray(qkv_b, np.float32)
    proj_w = np.asarray(proj_w, np.float32)
    proj_b = np.asarray(proj_b, np.float32)
    gn_w = np.asarray(gn_w, np.float32)
    gn_b = np.asarray(gn_b, np.float32)

    def chunked(a):  # [256, ...] -> [128, 2, ...]
        return np.ascontiguousarray(a.reshape(2, 128, *a.shape[1:]).transpose(
            1, 0, *range(2, a.ndim + 1)))

    wq = chunked(qkv_w[0:C].T.copy())            # [c_in, 2, c_out]
    wvf = chunked(qkv_w[2 * C:3 * C].T.copy())   # [c_in, 2, c_out]
    wpf = chunked(proj_w.T.copy())               # [c_in, 2, c_out]
    wkTb = chunked(qkv_w[C:2 * C].copy()).astype(bf16)   # [c_out, 2, c_in]
    wv8 = wvf.astype(fp8)
    wp8 = wpf.astype(fp8)
    rbias = proj_w @ qkv_b[2 * C:3 * C] + proj_b   # v-bias fold + proj bias
    kb_unused = np.zeros(C, np.float32)
    smalls = np.stack([qkv_b[0:C], kb_unused, gn_w, gn_b, rbias], axis=1)
    smalls = chunked(smalls)

    cidx = np.arange(C)
    ag_full = (cidx[:, None] // CPG == np.arange(G)[None, :]).astype(np.float32)
    ag = chunked(ag_full / CPG)                     # carries 1/8
    bg_full = ag_full * gn_w[:, None]               # carries gn_w
    bg = np.ascontiguousarray(
        bg_full.reshape(2, 128, G).transpose(2, 0, 1))  # [G, 2, 128]

    maps = []
    for core in range(8):
        b, half = core // 2, core % 2
        xf = x[b].reshape(C, HW)
        xh = np.ascontiguousarray(xf[:, half * NH:(half + 1) * NH])
        xT8 = np.ascontiguousarray(
            xf.T.reshape(MB, 128, C).transpose(1, 0, 2)).astype(fp8)
        maps.append({
            "x8": chunked(xf).astype(fp8),
            "x8h": chunked(xh).astype(fp8),
            "xT8": xT8, "xh": chunked(xh),
            "wq": wq, "wvf": wvf, "wpf": wpf,
            "wkTb": wkTb, "wv8": wv8, "wp8": wp8,
            "sb": smalls, "ag": ag, "bg": bg,
        })
    return maps


def kernel(x, gn_w, gn_b, qkv_w, qkv_b, proj_w, proj_b):
    import concourse.bass_utils as bu
    nc = build_nc()
    maps = _prep_maps(x, gn_w, gn_b, qkv_w, qkv_b, proj_w, proj_b)
    res = bu.run_bass_kernel_spmd(nc, maps, core_ids=list(range(8)))
    out = np.empty((B, C, HW), np.float32)
    for core in range(8):
        b, half = core // 2, core % 2
        o = res.results[core]["out"]                # [128, 2, NH]
        out[b, :, half * NH:(half + 1) * NH] = \
            o.transpose(1, 0, 2).reshape(C, NH)
    return out.reshape(B, C, 64, 64)
